# revision 46
# baseline (speedup 1.0000x reference)
"""Trainium2 Bass kernel for LlamaSwiftKV-style attention.

Full (unsharded) inputs in, full output out. Internally tensor-parallel
over 8 NeuronCores: core c owns kv-head c and q-heads 4c..4c+3, i.e. a
512-wide slice of the q/o projection feature dim. Each core computes a
partial output projection [HID, T]; the partials are summed on host.

All large operands (K, V, q_w, o_w, x, mask bias, probs, attn) travel
and multiply in bf16 with fp32 PSUM accumulation — the kernel is
HBM-DMA-bound (the DMA engines are an exclusive ~360 GB/s device), so
halving the bytes halves the runtime; rel err stays ~6e-3, inside the
2e-2 gate.

Schedule: the DMA device must never idle and the last transfers must
gate as little compute as possible. Every engine queue's issue order
is pinned with order-only dep edges (the tile scheduler otherwise
parks late-resolving instructions ahead of ready work and stalls the
in-order sequencers). Stream order: smalls, q_w (8), o_w (4),
per-batch [kT, V] for b=0..7 with batch 7's kT prefetched during
batch 6 and its V split 16+8+4+4 chunks, then stores for tokens
0..47, 48..55, and 56..63. Batch 7's scores/softmax-denominator
chain runs off kT before any V arrives, so the tail after the last
input DMA is just the final P@V chunks + normalize + an 8-column
o-proj + a 182 ns store.

The mask is usually the fixed causal last-Q pattern, which only
touches kv chunk 31: that "compact" program loads a single [128, 32]
bias tile and adds it to chunk 31 only, with 1/sqrt(D) pre-folded into
the host cos/sin tables so exp() reads the score PSUM directly. A
general-mask program (per-batch full bias, scalar_tensor_tensor) is
built lazily if the mask doesn't match.
"""

import sys

for _p in ("/opt/trn_rl_repo", "/root/.axon_site/_ro/trn_rl_repo"):
    if _p not in sys.path:
        sys.path.append(_p)

import numpy as np
import ml_dtypes

BF16 = ml_dtypes.bfloat16

B, Q, HID = 8, 8, 4096
H, KVH, D = 32, 8, 128
KV = 4096
ROPE_THETA = 10000.0
NCORES = 8
G = H // KVH            # 4 q-heads per kv-head (= per core)
FEAT = G * D            # 512 feature slice per core
T = B * Q               # 64 tokens
NCHUNK = KV // 128      # 32 kv chunks
HALF = D // 2
NHC = HID // 128        # 32 output-column chunks

_CACHE = {}

# PE p-state keep-warm fill counts for the batch-6/7 tail (see junk())
JUNK = (0, 0, 0, 0, 0)


def _build_program(compact_mask, JUNK=None):
    JUNK = JUNK or globals()["JUNK"]
    import concourse.bass as bass
    import concourse.tile as tile
    from concourse import bacc, mybir
    from concourse.masks import make_identity
    from concourse.tile_rust import add_dep_helper
    from contextlib import ExitStack

    f32 = mybir.dt.float32
    bf = mybir.dt.bfloat16
    nc = bacc.Bacc("TRN2", target_bir_lowering=False, debug=False)

    xT_d = nc.dram_tensor("xt", [128, HID // 128, T], bf, kind="ExternalInput")
    qwT_d = nc.dram_tensor("qwt", [HID, FEAT], bf, kind="ExternalInput")
    owT_d = nc.dram_tensor("owt", [FEAT, HID], bf, kind="ExternalInput")
    kT_d = nc.dram_tensor("kt", [B, D, KV], bf, kind="ExternalInput")
    # v pre-swizzled on host so every DMA is contiguous per partition:
    # v: [B, 128(p), 32(chunk), 128(d)]
    v_d = nc.dram_tensor("v", [B, 128, NCHUNK, D], bf, kind="ExternalInput")
    if compact_mask:
        # causal mask only affects kv chunk 31: one shared bias tile
        mb_d = nc.dram_tensor("mb", [128, G * Q], bf, kind="ExternalInput")
    else:
        mb_d = nc.dram_tensor("mb", [B, 128, NCHUNK, Q], bf, kind="ExternalInput")
    ones_d = nc.dram_tensor("ones", [128, 1], bf, kind="ExternalInput")
    cosb_d = nc.dram_tensor("cosb", [T, FEAT], bf, kind="ExternalInput")
    sinb_d = nc.dram_tensor("sinb", [T, FEAT], bf, kind="ExternalInput")
    # output stored transposed-and-swizzled [128(p), T, HID/128], t-major so
    # both store DMAs are contiguous
    out_d = nc.dram_tensor("out", [128, T, NHC], bf, kind="ExternalOutput")

    with tile.TileContext(nc) as tc, ExitStack() as ctx:
        const = ctx.enter_context(tc.tile_pool(name="const", bufs=1))
        qw_pool = ctx.enter_context(tc.tile_pool(name="qw", bufs=4))
        kt_pool = ctx.enter_context(tc.tile_pool(name="kt", bufs=3))
        v_pool = ctx.enter_context(tc.tile_pool(name="v", bufs=3))
        mb_pool = ctx.enter_context(tc.tile_pool(name="mbp", bufs=2))
        e_pool = ctx.enter_context(tc.tile_pool(name="e", bufs=2))
        small = ctx.enter_context(tc.tile_pool(name="small", bufs=4))
        rope_pool = ctx.enter_context(tc.tile_pool(name="rope", bufs=1))
        ps_s = ctx.enter_context(tc.tile_pool(name="ps_s", bufs=2, space="PSUM"))
        ps_o = ctx.enter_context(tc.tile_pool(name="ps_o", bufs=2, space="PSUM"))
        ps_d = ctx.enter_context(tc.tile_pool(name="ps_d", bufs=1, space="PSUM"))
        ps_b = ctx.enter_context(tc.tile_pool(name="ps_b", bufs=2, space="PSUM"))

        Exp = mybir.ActivationFunctionType.Exp
        Copy = mybir.ActivationFunctionType.Copy

        # in-order chain per engine queue: the DMA device is exclusive and
        # FIFO, so pinning each queue's issue order (order-only edges) plus
        # two cross-queue sync handoffs fixes the whole transfer schedule
        sp_chain = []

        def sp_dma(out, in_):
            dma = nc.sync.dma_start(out=out, in_=in_)
            if sp_chain:
                add_dep_helper(dma.ins, sp_chain[-1].ins, sync=False,
                               reason="SP queue order")
            sp_chain.append(dma)
            return dma

        # constants (xt first so the first big transfer starts ASAP)
        xt = const.tile([128, HID // 128, T], bf)
        sp_dma(xt, xT_d.ap())
        cosb = const.tile([T, FEAT], bf)
        sp_dma(cosb, cosb_d.ap())
        sinb = const.tile([T, FEAT], bf)
        sp_dma(sinb, sinb_d.ap())
        ones_kv = const.tile([128, 1], bf)
        sp_dma(ones_kv, ones_d.ap())
        if compact_mask:
            bias31 = const.tile([128, G * Q], bf)
            sp_dma(bias31, mb_d.ap())
        ident = const.tile([T, T], f32)
        make_identity(nc, ident)
        ones_bc = const.tile([1, 128], f32)
        nc.vector.memset(ones_bc, 1.0)


        # The tile scheduler is free to reorder instructions within an
        # engine queue, which can park an instruction whose deps resolve
        # late ahead of ready work, stalling the in-order sequencer. Chain
        # every PE/ACT/DVE instruction in data-availability order with
        # order-only edges.
        chain_st = {"on": True, "pe": None, "act": None, "dve": None}

        def _chain(res, eng):
            if chain_st["on"] and chain_st[eng] is not None:
                add_dep_helper(res.ins, chain_st[eng].ins, sync=False,
                               reason=f"{eng} issue order")
            chain_st[eng] = res
            return res

        def mm(*a, **k):
            return _chain(nc.tensor.matmul(*a, **k), "pe")

        def tpose(*a, **k):
            return _chain(nc.tensor.transpose(*a, **k), "pe")

        def act(*a, **k):
            return _chain(nc.scalar.activation(*a, **k), "act")

        def dve(fn, *a, **k):
            return _chain(fn(*a, **k), "dve")

        junk_t = ps_d.tile([T, T], f32, tag="junk")

        def junk(n):
            # deterministic PE p-state keep-warm: fill known dependency-wait
            # gaps so the tail matmuls run at the ramped clock (chained, so
            # placement is exact)
            for _ in range(n):
                tpose(junk_t, ident, ident)

        # ---- q projection: psum [64, 512] accumulated over 32 k-chunks
        # out[T, FEAT] = x @ q_w_slice.T: lhsT = xt chunk [k, T], rhs = qwT chunk
        q_ps = ps_b.tile([T, FEAT], f32, tag="misc")
        nkc = HID // 128
        QCH = 4
        qw_dmas = []
        for cgrp in range(nkc // QCH):
            qw_t = qw_pool.tile([128, QCH, FEAT], bf)
            qw_dma = nc.gpsimd.dma_start(
                out=qw_t,
                in_=qwT_d.ap()
                .rearrange("(c p) f -> p c f", p=128)[
                    :, QCH * cgrp : QCH * (cgrp + 1), :
                ],
            )
            if qw_dmas:
                add_dep_helper(qw_dma.ins, qw_dmas[-1].ins, sync=False,
                               reason="Pool queue order")
            qw_dmas.append(qw_dma)
            for i in range(QCH):
                c = QCH * cgrp + i
                mm(
                    q_ps, xt[:, c, :], qw_t[:, i, :],
                    start=(c == 0), stop=(c == nkc - 1),
                )

        # ---- o_w stream: 4 quarters on the Activation queue, chained to
        # slot into the DMA device right behind the q_w stream
        ow_t = const.tile([128, G, HID], bf)
        ow_dmas = []
        for qi in range(4):
            owq = HID // 4
            ow_dma = nc.scalar.dma_start(
                out=ow_t[:, :, qi * owq : (qi + 1) * owq],
                in_=owT_d.ap().rearrange("(g p) n -> p g n", p=128)[
                    :, :, qi * owq : (qi + 1) * owq
                ],
            )
            if qi == 0:
                add_dep_helper(
                    ow_dma.ins,
                    qw_dmas[5].ins,
                    sync=True,
                    reason="o_w stream follows q_w stream",
                )
            else:
                add_dep_helper(ow_dma.ins, ow_dmas[-1].ins, sync=False,
                               reason="ACT queue order")
            ow_dmas.append(ow_dma)

        # ---- RoPE on the free axis (feat = g*128 + d); 1/sqrt(D) is folded
        # into the host cos/sin tables, so scores psums are pre-scaled
        qv = q_ps.rearrange("t (g h d) -> t g h d", g=G, h=2)
        rot = rope_pool.tile([T, G, 2, HALF], f32)
        dve(nc.vector.tensor_copy, rot[:, :, 0, :], qv[:, :, 1, :])
        dve(nc.vector.tensor_copy, rot[:, :, 1, :], qv[:, :, 0, :])
        q_rope = rope_pool.tile([T, FEAT], f32)
        dve(nc.vector.tensor_mul, q_rope, q_ps, cosb)
        rot_f = rot.rearrange("t g h d -> t (g h d)")
        dve(nc.vector.tensor_mul, rot_f, rot_f, sinb)
        dve(nc.vector.tensor_add, q_rope, q_rope, rot_f)

        # ---- transpose each head -> qT [128(d), G, 64(b,q)] (bf16)
        qT = const.tile([128, G, T], bf)
        for g in range(G):
            tp = ps_b.tile([128, T], f32, tag="misc")
            tpose(tp, q_rope[:, g * 128 : (g + 1) * 128], ident)
            dve(nc.vector.tensor_copy, qT[:, g, :], tp)

        # attention output (transposed, normalized) [128(d), G, 64(b,q)]
        attnT = const.tile([128, G, T], bf)
        # o-proj output tile, t-major [128(p), 64(t), 32(hc)]
        ot = const.tile([128, T, NHC], bf)

        # ---------- per-batch emission helpers ----------

        def emit_kv_dmas(b, split_v=False):
            kt_t = kt_pool.tile([128, KV], bf)
            kt0 = sp_dma(kt_t[:, : KV // 2], kT_d.ap()[b][:, : KV // 2])
            sp_dma(kt_t[:, KV // 2 :], kT_d.ap()[b][:, KV // 2 :])
            return kt_t, kt0

        def emit_mb_dma(b):
            mb_t = mb_pool.tile([128, NCHUNK, Q], bf)
            sp_dma(mb_t, mb_d.ap()[b])
            return mb_t

        def emit_v_dmas(b, split_v):
            v_t = v_pool.tile([128, NCHUNK, D], bf)
            if split_v:
                sp_dma(v_t[:, : NCHUNK // 2, :], v_d.ap()[b][:, : NCHUNK // 2, :])
                sp_dma(v_t[:, NCHUNK // 2 :, :], v_d.ap()[b][:, NCHUNK // 2 :, :])
            else:
                sp_dma(v_t, v_d.ap()[b])
            return v_t

        def emit_scores_exp(b, kt_t, mb_t):
            """scores + exp -> e_t [128, 32, 32] bf16."""
            e_t = e_pool.tile([128, NCHUNK, G * Q], bf)
            if not compact_mask:
                # expand mask [128, 32, 8] -> [128, 32, 32] (walrus APs are
                # 3-dim, so no stride-0 4D broadcast in the bias op)
                mbx = e_pool.tile([128, NCHUNK, G * Q], f32, tag="mbx")
                dve(nc.vector.tensor_copy, mbx[:, :, 0:Q], mb_t)
                dve(nc.vector.tensor_copy, mbx[:, :, Q : 2 * Q], mbx[:, :, 0:Q])
                dve(nc.vector.tensor_copy, mbx[:, :, 2 * Q :], mbx[:, :, 0 : 2 * Q])
            for cg in range(2):
                s_ps = ps_s.tile([128, 16 * G * Q], f32)
                for cc in range(16):
                    c = cg * 16 + cc
                    mm(
                        s_ps[:, cc * 32 : (cc + 1) * 32],
                        kt_t[:, c * 128 : (c + 1) * 128],
                        qT[:, :, b * Q : (b + 1) * Q],
                        start=True,
                        stop=True,
                    )
                if compact_mask:
                    if cg == 0:
                        act(
                            e_t[:, 0:16, :].rearrange("p c j -> p (c j)"),
                            s_ps, Exp,
                        )
                    else:
                        # chunks 16..30 are bias-free; chunk 31 gets the
                        # shared causal bias
                        dve(nc.vector.tensor_add,
                            s_ps[:, 15 * 32 :], s_ps[:, 15 * 32 :], bias31)
                        act(
                            e_t[:, 16:31, :].rearrange("p c j -> p (c j)"),
                            s_ps[:, : 15 * 32], Exp,
                        )
                        act(e_t[:, 31, :], s_ps[:, 15 * 32 :], Exp)
                else:
                    dve(nc.vector.tensor_add,
                        s_ps,
                        s_ps,
                        mbx[:, cg * 16 : (cg + 1) * 16, :].rearrange(
                            "p c j -> p (c j)"
                        ))
                    act(
                        e_t[:, cg * 16 : (cg + 1) * 16, :].rearrange(
                            "p c j -> p (c j)"
                        ),
                        s_ps,
                        Exp,
                    )
            return e_t

        def emit_den(b, e_t, j_mid=0, j_end=0):
            """denominator -> bc_sb [128, 32] f32 in SBUF."""
            d_ps = ps_d.tile([1, 16 * G * Q], f32)
            mm(
                d_ps, ones_kv,
                e_t[:, 0:16, :].rearrange("p c j -> p (c j)"),
                start=True, stop=False,
            )
            junk(j_mid)
            mm(
                d_ps, ones_kv,
                e_t[:, 16:32, :].rearrange("p c j -> p (c j)"),
                start=False, stop=True,
            )
            junk(j_end)
            den = small.tile([1, G * Q], f32)
            dve(nc.vector.reduce_sum,
                den,
                d_ps.rearrange("p (c j) -> p j c", c=16),
                axis=mybir.AxisListType.X,
            )
            rec = small.tile([1, G * Q], f32)
            dve(nc.vector.reciprocal, rec, den)
            bc_ps = ps_b.tile([128, G * Q], f32, tag="misc", name=f"bc_{b}")
            mm(bc_ps, ones_bc, rec, start=True, stop=True)
            # stage in SBUF so the normalize reads only one PSUM operand
            bc_sb = small.tile([128, G * Q], f32)
            act(bc_sb, bc_ps, Copy)
            return bc_sb

        def emit_pv(b, v_t, e_t, chunks, o_ps):
            for c in chunks:
                mm(
                    o_ps, v_t[:, c, :], e_t[:, c, :],
                    start=(c == 0), stop=(c == NCHUNK - 1),
                )

        def emit_norm_oproj(b, o_ps, bc_sb, split_ot=False, j_pre=0):
            dve(nc.vector.tensor_mul,
                attnT[:, :, b * Q : (b + 1) * Q],
                o_ps.rearrange("p (g q) -> p g q", g=G),
                bc_sb.rearrange("p (g q) -> p g q", g=G),
            )
            junk(j_pre)
            op_ps = ps_b.tile([128, NHC, Q], f32, tag="misc", name=f"op_{b}")
            for hc in range(NHC):
                for g in range(G):
                    mm(
                        op_ps[:, hc, :],
                        ow_t[:, g, hc * 128 : (hc + 1) * 128],
                        attnT[:, g, b * Q : (b + 1) * Q],
                        start=(g == 0),
                        stop=(g == G - 1),
                    )
            if split_ot:
                half = NHC // 2
                act(
                    ot[:, b * Q : (b + 1) * Q, 0:half].rearrange(
                        "p t c -> p c t"),
                    op_ps[:, 0:half, :],
                    Copy,
                )
                act(
                    ot[:, b * Q : (b + 1) * Q, half:].rearrange(
                        "p t c -> p c t"),
                    op_ps[:, half:, :],
                    Copy,
                )
            else:
                act(
                    ot[:, b * Q : (b + 1) * Q, :].rearrange("p t c -> p c t"),
                    op_ps,
                    Copy,
                )
            return op_ps

        # ---------- batches 0..5: straight pipeline ----------
        kt_b0_dma0 = None
        for b in range(B - 2):
            kt_t, kt0 = emit_kv_dmas(b)
            if b == 0:
                kt_b0_dma0 = kt0
            mb_t = emit_mb_dma(b) if not compact_mask else None
            v_t = emit_v_dmas(b, split_v=False)
            e_t = emit_scores_exp(b, kt_t, mb_t)
            bc_sb = emit_den(b, e_t)
            o_ps = ps_o.tile([128, G * Q], f32, tag="o_ps")
            emit_pv(b, v_t, e_t, range(NCHUNK), o_ps)
            emit_norm_oproj(b, o_ps, bc_sb)

        # keep the kv stream behind the weight streams
        add_dep_helper(
            kt_b0_dma0.ins,
            ow_dmas[1].ins,
            sync=True,
            reason="kv stream follows o_w stream",
        )

        # ---------- batches 6 and 7: software-pipelined tail ----------
        b6, b7 = B - 2, B - 1
        kt6_t, _ = emit_kv_dmas(b6)
        kt7_t, _ = emit_kv_dmas(b7)          # prefetched during batch 6
        mb6_t = emit_mb_dma(b6) if not compact_mask else None
        v6_t = emit_v_dmas(b6, split_v=False)
        mb7_t = emit_mb_dma(b7) if not compact_mask else None
        # batch 7's V in three pieces: 16 + 8 + 8 chunks, so P@V trails the
        # transfers and only the last 8 chunks sit past the final input DMA
        v7_t = v_pool.tile([128, NCHUNK, D], bf)
        sp_dma(v7_t[:, 0:16, :], v_d.ap()[b7][:, 0:16, :])
        sp_dma(v7_t[:, 16:24, :], v_d.ap()[b7][:, 16:24, :])
        sp_dma(v7_t[:, 24:28, :], v_d.ap()[b7][:, 24:28, :])
        sp_dma(v7_t[:, 28:32, :], v_d.ap()[b7][:, 28:32, :])

        # batch 6 + 7 scores/exp/denominator all run off kt, before any V
        e6_t = emit_scores_exp(b6, kt6_t, mb6_t)
        bc6_sb = emit_den(b6, e6_t)
        e7_t = emit_scores_exp(b7, kt7_t, mb7_t)
        bc7_sb = emit_den(b7, e7_t, j_mid=JUNK[0], j_end=JUNK[1])

        junk(JUNK[2])
        o6_ps = ps_o.tile([128, G * Q], f32, tag="o_ps")
        emit_pv(b6, v6_t, e6_t, range(NCHUNK), o6_ps)
        o7_ps = ps_o.tile([128, G * Q], f32, tag="o_ps")
        emit_pv(b7, v7_t, e7_t, range(16), o7_ps)
        emit_norm_oproj(b6, o6_ps, bc6_sb, j_pre=JUNK[3])
        emit_pv(b7, v7_t, e7_t, range(16, 24), o7_ps)
        emit_pv(b7, v7_t, e7_t, range(24, 28), o7_ps)
        emit_pv(b7, v7_t, e7_t, range(28, NCHUNK), o7_ps)
        emit_norm_oproj(b7, o7_ps, bc7_sb, j_pre=JUNK[4])

        # ---- stores: tokens 0..47 right after the last input transfer,
        # then batch 6's columns, then batch 7's as a final 182 ns store
        T1 = (B - 2) * Q
        T2 = (B - 1) * Q
        sp_dma(out_d.ap()[:, 0:T1, :], ot[:, 0:T1, :])
        sp_dma(out_d.ap()[:, T1:T2, :], ot[:, T1:T2, :])
        sp_dma(out_d.ap()[:, T2:T, :], ot[:, T2:T, :])

    nc.compile()
    return nc


def _get_program(compact_mask=True):
    key = ("nc", compact_mask)
    if key not in _CACHE:
        _CACHE[key] = _build_program(compact_mask)
    return _CACHE[key]


def _causal_bias31():
    """Bias tile for kv chunk 31 under the causal last-Q mask: [128, G*Q]."""
    j = KV - 128 + np.arange(128)
    qpos = KV - Q + np.arange(Q)
    bias = np.where(j[:, None] > qpos[None, :], np.float32(-10000.0),
                    np.float32(0.0))                      # [128, Q]
    return np.ascontiguousarray(np.tile(bias, (1, G))).astype(BF16)


def _is_causal_mask(mask):
    qpos = KV - Q + np.arange(Q)
    causal = np.arange(KV)[None, :] > qpos[:, None]       # [Q, KV]
    return bool(
        np.array_equal(np.asarray(mask),
                       np.broadcast_to(causal[None, None], (B, 1, Q, KV)))
    )


def _host_prep(hidden_states, position_ids, key_cache, value_cache, attention_mask, q_w, o_w):
    """Build the per-core input maps (all host-side layout marshaling)."""
    x = np.asarray(hidden_states, np.float32).reshape(T, HID)
    xT = np.ascontiguousarray(
        x.T.reshape(HID // 128, 128, T).transpose(1, 0, 2)
    ).astype(BF16)

    pos = np.asarray(position_ids)
    idx = int(np.argmax(pos[0].astype(np.int32)))
    pid = pos[:, idx].astype(np.float32)                      # [B]
    inv_freq = 1.0 / (ROPE_THETA ** (np.arange(0, HALF, dtype=np.float32) / HALF))
    ang = pid[:, None] * inv_freq[None, :]                    # [B, 64]
    emb = np.concatenate([ang, ang], axis=1)                  # [B, 128]
    scl = np.float32(1.0 / np.sqrt(D))   # fold 1/sqrt(D) into the rope tables
    cos_b = (np.cos(emb) * scl).astype(np.float32)
    sin_b = (np.sin(emb) * scl).astype(np.float32)
    sign = np.concatenate(
        [-np.ones(HALF, np.float32), np.ones(HALF, np.float32)]
    )
    sin_s = sin_b * sign[None, :]
    cosb = np.ascontiguousarray(
        np.tile(np.repeat(cos_b, Q, axis=0), (1, G))
    ).astype(BF16)
    sinb = np.ascontiguousarray(
        np.tile(np.repeat(sin_s, Q, axis=0), (1, G))
    ).astype(BF16)

    compact = _is_causal_mask(attention_mask)
    if compact:
        mb_host = _causal_bias31()
    else:
        mask = np.asarray(attention_mask)[:, 0]               # [B, Q, KV] bool
        mbias = np.where(mask, np.float32(-10000.0), np.float32(0.0))
        mbT = mbias.transpose(0, 2, 1)                        # [B, KV, Q]
        mb_host = np.ascontiguousarray(
            mbT.reshape(B, NCHUNK, 128, Q).transpose(0, 2, 1, 3)
        ).astype(BF16)

    kc = np.asarray(key_cache, np.float32)
    vc = np.asarray(value_cache, np.float32)
    qw = np.asarray(q_w, np.float32)
    ow = np.asarray(o_w, np.float32)

    ones_col = np.ones((128, 1), BF16)
    in_maps = []
    for c in range(NCORES):
        kT = np.ascontiguousarray(kc[:, c].transpose(0, 2, 1)).astype(BF16)
        v_sw = np.ascontiguousarray(
            vc[:, c].reshape(B, NCHUNK, 128, D).transpose(0, 2, 1, 3)
        ).astype(BF16)                                             # [B,128,32,128]
        qwT = np.ascontiguousarray(
            qw[c * FEAT : (c + 1) * FEAT, :].T
        ).astype(BF16)                                             # [HID, 512]
        owT = np.ascontiguousarray(
            ow[:, c * FEAT : (c + 1) * FEAT].T
        ).astype(BF16)                                             # [512, HID]
        in_maps.append(
            {
                "ones": ones_col,
                "xt": xT,
                "qwt": qwT,
                "owt": owT,
                "kt": kT,
                "v": v_sw,
                "mb": mb_host,
                "cosb": cosb,
                "sinb": sinb,
            }
        )
    return compact, in_maps


def kernel(
    hidden_states,
    position_ids,
    key_cache,
    value_cache,
    attention_mask,
    q_w,
    o_w,
    _trace=False,
):
    from concourse.bass_utils import run_bass_kernel_spmd

    compact, in_maps = _host_prep(
        hidden_states, position_ids, key_cache, value_cache, attention_mask, q_w, o_w
    )
    nc = _get_program(compact)
    res = run_bass_kernel_spmd(nc, in_maps, list(range(NCORES)), trace=_trace)
    _CACHE["last_result"] = res
    out = np.zeros((128, T, NHC), np.float32)
    for r in res.results:
        out += np.asarray(r["out"], dtype=np.float32)
    # [128(p), 64(t), 32(c)] -> [B, Q, HID]: out[b,q,c*128+p]
    return np.ascontiguousarray(
        out.transpose(1, 2, 0).reshape(B, Q, HID)
    )
